# revision 18
# baseline (speedup 1.0000x reference)
"""VQ codebook quantizer (nn_BaseQuantizer) on 8 Trainium2 NeuronCores.

Problem (hardcoded shapes):
  x        [8, 4096, 128] f32
  codebook [128, 1024]    f32
  returns (x_quantized [8,4096,128] f32, codebook_index [8,4096] i32,
           inner_loss scalar f32)

Strategy: data-parallel over batch — core c handles x[c] (4096 tokens).
x and codebook are scaled by S=32 and split into fp16 hi+lo halves on the
host (argmin is invariant to the common scale). Per 128-token tile:
  PE   : scores = xh.ch + xh.cl + xl.ch     (6 fp16 matmuls, N=512;
         ~1e-5 reconstruction error at a ~37x flip margin)
  DVE  : one fused pass  argmax_k(scores - S^2|c_k|^2/2)  == argmin_k dist
         (custom DVE op: running-max scan + select(eq) + MAX-accum of Idx)
  ACT  : cast the argmax to int32
  DMA  : indirect-gather codebook.T[idx] -> x_quantized tile (SWDGE)
Outputs use DMA-friendly layouts (idx [128, 32] transposed; xq permuted
[128, NG, GRP*128] with 4KB-contiguous runs) and are un-permuted on the
host. The scalar commitment loss is reduced on host from the outputs
(same formula as the reference: mean((xq - x)^2)).
"""
import sys

for _p in ("/opt/trn_rl_repo",):
    if _p not in sys.path:
        sys.path.insert(0, _p)

import numpy as np

import concourse.bass as bass
import concourse.bacc as bacc
import concourse.mybir as mybir
import concourse.tile as tile
from concourse import dve_ops
from concourse.bass_utils import run_bass_kernel_spmd
from concourse.dve_ops import DveOp
from concourse.dve_spec import (
    AluOp,
    Idx,
    One,
    Spec,
    Src0,
    Src1,
    Zero,
    eq,
    lower,
    scan,
    select,
)
from concourse.dve_uop import DveOpSpec

B, L, D, K = 8, 4096, 128, 1024
N_CORES = 8
TILE = 128                # tokens per matmul/argmax tile
NT = L // TILE            # 32 tiles per core
GRP = 8                   # tiles per load/store group
NG = NT // GRP            # 4 groups
S = 32.0                  # common scale before the fp16 split


# ---------------------------------------------------------------- DVE op --- #
def _ref_vq_argmax(in0, in1, c0, c1, c2):
    v = in0.astype(np.float32) - in1.astype(np.float32)
    r = np.maximum.accumulate(v, axis=-1)
    idx = np.arange(v.shape[-1], dtype=np.float32)
    body = np.where(v == r, idx, -1.0).astype(np.float32)
    acc = body.max(axis=-1).reshape(v.shape[0], -1)[:, -1:]
    return body, acc


def _make_argmax_op() -> DveOp:
    """Register (once) a fused single-pass argmax-of-difference DVE op."""
    name = "VQ_ARGMAX_DIFF"
    for op in dve_ops.OPS:
        if op.name == name:
            return op
    v = Src0 - Src1
    r = scan(AluOp.MAX, v)
    body = select(eq(v, r), Idx, Zero - One)
    spec = Spec(body=body, accum=AluOp.MAX, reference=_ref_vq_argmax)
    row = dve_ops._CUSTOM_DVE_ROW_BASE + len(dve_ops.OPS)
    assert row < 0x20, "custom DVE row overflow"
    shas = {
        ver: DveOpSpec(
            name=name, opcode=row, uops=lower(spec, ver=ver), rd1_en=True
        ).sha(ver)
        for ver in ("v3", "v4")
    }
    op = DveOp(name, spec, subdim=False, uops_sha=shas)
    dve_ops.OPS.append(op)
    dve_ops.CUSTOM_DVE_SPECS[name] = spec
    dve_ops._SUB_OPCODE_FOR_NAME[name] = row
    return op


# ----------------------------------------------------------------- kernel --- #
_CACHE: dict = {}


def _build():
    if "nc" in _CACHE:
        return _CACHE["nc"]
    argmax_op = _make_argmax_op()

    nc = bacc.Bacc("TRN2", target_bir_lowering=False, debug=False)
    # chunk q of xThl: cols [2*CW*q, 2*CW*q+CW) = xh, [2*CW*q+CW, ...) = xl
    xThl_ext = nc.dram_tensor(
        "xThl", [D, 2 * L], mybir.dt.float16, kind="ExternalInput"
    )
    cbh_ext = nc.dram_tensor("cbh", [D, K], mybir.dt.float16, kind="ExternalInput")
    cbl_ext = nc.dram_tensor("cbl", [D, K], mybir.dt.float16, kind="ExternalInput")
    cbsq_ext = nc.dram_tensor(
        "cbsq", [1, K], mybir.dt.float32, kind="ExternalInput"
    )
    cbT_ext = nc.dram_tensor("cbT", [K, D], mybir.dt.float32, kind="ExternalInput")
    # idx, transposed: [p, t] = index of token t*TILE+p
    idxT_ext = nc.dram_tensor("idxT", [TILE, NT], mybir.dt.int32, kind="ExternalOutput")
    # xq, permuted: [p, g, j*D + d] = token (g*GRP+j)*TILE + p
    xqP_ext = nc.dram_tensor(
        "xqP", [TILE, NG, GRP * D], mybir.dt.float32, kind="ExternalOutput"
    )

    GW = GRP * TILE       # tokens per store group (1024)
    CT = 4                # tiles per load chunk
    CW = CT * TILE        # tokens per load chunk (512)

    with tile.TileContext(nc) as tc:
        with (
            tc.tile_pool(name="static", bufs=1) as static_pool,
            tc.tile_pool(name="loads", bufs=8) as load_pool,
            tc.tile_pool(name="work", bufs=2) as work_pool,
            tc.tile_pool(name="small", bufs=8) as small_pool,
            tc.tile_pool(name="out", bufs=3) as out_pool,
            tc.tile_pool(name="psum", bufs=3, space="PSUM") as psum_pool,
            tc.tile_pool(name="psum1", bufs=1, space="PSUM") as psum1_pool,
        ):
            cbh_t = static_pool.tile([D, K], mybir.dt.float16)
            cbl_t = static_pool.tile([D, K], mybir.dt.float16)
            cbsq_row = static_pool.tile([1, K], mybir.dt.float32)
            ones_t = static_pool.tile([1, TILE], mybir.dt.float32)
            cbsq_t = static_pool.tile([128, K], mybir.dt.float32)
            idx_acc = static_pool.tile([TILE, NT], mybir.dt.int32)
            nc.sync.dma_start(out=cbsq_row[:], in_=cbsq_ext[:])
            nc.sync.dma_start(out=cbh_t[:], in_=cbh_ext[:])

            # x loads: tiny first chunk (1 tile) so compute starts early,
            # then 4-tile chunks; all resident (bufs=8), all on the scalar
            # HWDGE queue, issued before anything else enters that FIFO.
            bounds = [0, TILE] + [CW * q for q in range(1, L // CW)] + [L]
            xchunks = []
            for q in range(len(bounds) - 1):
                lo, hi = bounds[q], bounds[q + 1]
                xThl_c = load_pool.tile([D, 2 * (hi - lo)], mybir.dt.float16,
                                        tag=f"xc{q}")
                nc.scalar.dma_start(
                    out=xThl_c[:], in_=xThl_ext[:, 2 * lo : 2 * hi]
                )
                xchunks.append((lo, hi, xThl_c))
                if q == 0:
                    # cbl rides the scalar queue right after the tiny first
                    # x chunk, in parallel with cbh on the sync queue
                    nc.scalar.dma_start(out=cbl_t[:], in_=cbl_ext[:])

            # broadcast cbsq to all 128 partitions: ones.T @ cbsq_row via PE,
            # PSUM->SBUF copy on the DVE (idle until the first argmax anyway)
            nc.vector.memset(ones_t[:], 1.0)
            cbsq_ps = psum1_pool.tile([128, K], mybir.dt.float32)
            # PE warm-up: ~3.5us of dummy matmuls with no DMA dependency so the
            # HAM clock-gate reaches 8/8 before the first real tile; results
            # land in the cbsq bank and are overwritten right after.
            wu_w = static_pool.tile([D, TILE], mybir.dt.float16)
            wu_m = static_pool.tile([D, 512], mybir.dt.float16)
            nc.vector.memset(wu_w[:], 0.0)
            nc.vector.memset(wu_m[:], 0.0)
            for _ in range(7):
                nc.tensor.matmul(
                    out=cbsq_ps[:, 0:512], lhsT=wu_w[:], rhs=wu_m[:],
                    start=True, stop=True,
                )
            for h in range(K // 512):
                hs = slice(h * 512, (h + 1) * 512)
                nc.tensor.matmul(
                    out=cbsq_ps[:, hs], lhsT=ones_t[:], rhs=cbsq_row[:, hs],
                    start=True, stop=True,
                )
            nc.vector.tensor_copy(out=cbsq_t[:], in_=cbsq_ps[:])

            def x_slices(t):
                for lo, hi, c in xchunks:
                    if lo <= t * TILE < hi:
                        off = t * TILE - lo
                        w = hi - lo
                        return (c[:, off : off + TILE],
                                c[:, w + off : w + off + TILE])
                raise AssertionError(t)

            for g in range(NG):
                xq_big = out_pool.tile([TILE, GRP, D], mybir.dt.float32)
                for j in range(GRP):
                    t = g * GRP + j
                    xh, xl = x_slices(t)
                    scores = psum_pool.tile([TILE, K], mybir.dt.float32)
                    # stationary operand changes once per tile: xh then xl
                    for h in range(K // 512):
                        hs = slice(h * 512, (h + 1) * 512)
                        nc.tensor.matmul(
                            out=scores[:, hs], lhsT=xh, rhs=cbh_t[:, hs],
                            start=True, stop=False,
                        )
                    for h in range(K // 512):
                        hs = slice(h * 512, (h + 1) * 512)
                        nc.tensor.matmul(
                            out=scores[:, hs], lhsT=xh, rhs=cbl_t[:, hs],
                            start=False, stop=False,
                        )
                    for h in range(K // 512):
                        hs = slice(h * 512, (h + 1) * 512)
                        nc.tensor.matmul(
                            out=scores[:, hs], lhsT=xl, rhs=cbh_t[:, hs],
                            start=False, stop=True,
                        )
                    scratch = work_pool.tile([TILE, K], mybir.dt.float32)
                    amax_f = small_pool.tile([TILE, 1], mybir.dt.float32)
                    nc.vector._custom_dve(
                        argmax_op,
                        out=scratch[:],
                        in0=scores[:],
                        in1=cbsq_t[:],
                        accum_out=amax_f[:],
                    )
                    # cast f32 -> i32 on the (mostly idle) scalar engine
                    nc.scalar.copy(out=idx_acc[:, t : t + 1], in_=amax_f[:])

                    nc.gpsimd.indirect_dma_start(
                        out=xq_big[:, j, :],
                        out_offset=None,
                        in_=cbT_ext[:],
                        in_offset=bass.IndirectOffsetOnAxis(
                            ap=idx_acc[:, t : t + 1], axis=0
                        ),
                    )
                    if g == NG - 1:
                        nc.sync.dma_start(
                            out=xqP_ext[:, g, j * D : (j + 1) * D],
                            in_=xq_big[:, j : j + 1, :],
                        )
                    elif j % 2 == 1:
                        nc.sync.dma_start(
                            out=xqP_ext[:, g, (j - 1) * D : (j + 1) * D],
                            in_=xq_big[:, j - 1 : j + 1, :],
                        )
            nc.sync.dma_start(out=idxT_ext[:], in_=idx_acc[:])

    nc.compile()
    _CACHE["nc"] = nc
    return nc


def _ensure_ntff_hook():
    """The image's antenv lacks axon_hooks; synthesize it so trace=True works."""
    try:
        from antenv.axon_hooks import get_axon_ntff_profile_hook  # noqa: F401

        return
    except ImportError:
        pass
    import types

    import antenv
    from trn_agent_boot.trn_boot import _ntff_profile_via_ctypes

    hook = _ntff_profile_via_ctypes("/opt/axon/libaxon_pjrt.so")
    mod = types.ModuleType("antenv.axon_hooks")
    mod._hook = hook
    mod.get_axon_ntff_profile_hook = lambda: mod._hook
    mod.set_axon_ntff_profile_hook = lambda h: setattr(mod, "_hook", h)
    sys.modules["antenv.axon_hooks"] = mod
    antenv.axon_hooks = mod


def _run(x: np.ndarray, codebook: np.ndarray, trace: bool = False):
    """Shard, run on 8 cores, reassemble. Returns (xq, idx, results_obj)."""
    if trace:
        _ensure_ntff_hook()
    nc = _build()
    x = np.ascontiguousarray(x, dtype=np.float32)
    codebook = np.ascontiguousarray(codebook, dtype=np.float32)
    cbs = codebook * np.float32(S)
    cbh = cbs.astype(np.float16)
    cbl = (cbs - cbh.astype(np.float32)).astype(np.float16)
    # scores come out scaled by S^2; match with S^2 * |c|^2 / 2
    cbsq_half = (cbs.astype(np.float64) ** 2).sum(axis=0) / 2.0
    cbsq_row = cbsq_half.astype(np.float32)[None, :]
    cbT = np.ascontiguousarray(codebook.T)  # unscaled: gather output values

    CW = 4 * TILE
    bounds = [0, TILE] + [CW * q for q in range(1, L // CW)] + [L]
    in_maps = []
    for c in range(N_CORES):
        xs = np.ascontiguousarray(x[c].T) * np.float32(S)
        xh = xs.astype(np.float16)
        xl = (xs - xh.astype(np.float32)).astype(np.float16)
        xThl = np.empty((D, 2 * L), dtype=np.float16)
        for q in range(len(bounds) - 1):
            lo, hi = bounds[q], bounds[q + 1]
            w = hi - lo
            xThl[:, 2 * lo : 2 * lo + w] = xh[:, lo:hi]
            xThl[:, 2 * lo + w : 2 * hi] = xl[:, lo:hi]
        in_maps.append(
            {
                "xThl": xThl,
                "cbh": cbh,
                "cbl": cbl,
                "cbsq": cbsq_row,
                "cbT": cbT,
            }
        )
    res = run_bass_kernel_spmd(nc, in_maps, list(range(N_CORES)), trace=trace)

    xq = np.empty((B, L, D), dtype=np.float32)
    idx = np.empty((B, L), dtype=np.int32)
    for c in range(N_CORES):
        idxT = res.results[c]["idxT"]              # [TILE, NT]
        idx[c] = idxT.T.reshape(L)
        xqP = res.results[c]["xqP"]                # [TILE, NG, GRP*D]
        xq[c] = (
            xqP.reshape(TILE, NG, GRP, D)
            .transpose(1, 2, 0, 3)
            .reshape(L, D)
        )
    return xq, idx, res


def kernel(x: np.ndarray, codebook: np.ndarray):
    xq, idx, _ = _run(x, codebook, trace=False)
    x = np.asarray(x, dtype=np.float32)
    # commitment loss: mean((stop_grad(xq) - x)^2), accumulated in f64
    diff = xq - x
    inner_loss = np.float32(np.mean(diff.astype(np.float64) ** 2))
    # straight-through estimator: x + stop_grad(xq - x) — reproduce the
    # reference's forward arithmetic exactly
    x_quantized = x + diff
    return x_quantized, idx, inner_loss


# revision 19
# speedup vs baseline: 1.0651x; 1.0651x over previous
"""VQ codebook quantizer (nn_BaseQuantizer) on 8 Trainium2 NeuronCores.

Problem (hardcoded shapes):
  x        [8, 4096, 128] f32
  codebook [128, 1024]    f32
  returns (x_quantized [8,4096,128] f32, codebook_index [8,4096] i32,
           inner_loss scalar f32)

Strategy: data-parallel over batch — core c handles x[c] (4096 tokens).
x and codebook are scaled by S=32 and split into fp16 hi+lo halves on the
host (argmin is invariant to the common scale). Per 128-token tile:
  PE   : scores = xh.ch + xh.cl + xl.ch     (6 fp16 matmuls, N=512;
         ~1e-5 reconstruction error at a ~37x flip margin)
  DVE  : one fused pass  argmax_k(scores - S^2|c_k|^2/2)  == argmin_k dist
         (custom DVE op: running-max scan + select(eq) + MAX-accum of Idx)
  ACT  : cast the argmax to int32
  DMA  : indirect-gather codebook.T[idx] -> x_quantized tile (SWDGE)
Outputs use DMA-friendly layouts (idx [128, 32] transposed; xq permuted
[128, NG, GRP*128] with 4KB-contiguous runs) and are un-permuted on the
host. The scalar commitment loss is reduced on host from the outputs
(same formula as the reference: mean((xq - x)^2)).
"""
import sys

for _p in ("/opt/trn_rl_repo",):
    if _p not in sys.path:
        sys.path.insert(0, _p)

import numpy as np

import concourse.bass as bass
import concourse.bacc as bacc
import concourse.mybir as mybir
import concourse.tile as tile
from concourse import dve_ops
from concourse.bass_utils import run_bass_kernel_spmd
from concourse.dve_ops import DveOp
from concourse.dve_spec import (
    AluOp,
    Idx,
    One,
    Spec,
    Src0,
    Src1,
    Zero,
    eq,
    lower,
    scan,
    select,
)
from concourse.dve_uop import DveOpSpec

B, L, D, K = 8, 4096, 128, 1024
N_CORES = 8
TILE = 128                # tokens per matmul/argmax tile
NT = L // TILE            # 32 tiles per core
GRP = 8                   # tiles per load/store group
NG = NT // GRP            # 4 groups
S = 32.0                  # common scale before the fp16 split


# ---------------------------------------------------------------- DVE op --- #
def _ref_vq_argmax(in0, in1, c0, c1, c2):
    v = in0.astype(np.float32) - in1.astype(np.float32)
    r = np.maximum.accumulate(v, axis=-1)
    idx = np.arange(v.shape[-1], dtype=np.float32)
    body = np.where(v == r, idx, -1.0).astype(np.float32)
    acc = body.max(axis=-1).reshape(v.shape[0], -1)[:, -1:]
    return body, acc


def _make_argmax_op() -> DveOp:
    """Register (once) a fused single-pass argmax-of-difference DVE op."""
    name = "VQ_ARGMAX_DIFF"
    for op in dve_ops.OPS:
        if op.name == name:
            return op
    v = Src0 - Src1
    r = scan(AluOp.MAX, v)
    body = select(eq(v, r), Idx, Zero - One)
    spec = Spec(body=body, accum=AluOp.MAX, reference=_ref_vq_argmax)
    row = dve_ops._CUSTOM_DVE_ROW_BASE + len(dve_ops.OPS)
    assert row < 0x20, "custom DVE row overflow"
    shas = {
        ver: DveOpSpec(
            name=name, opcode=row, uops=lower(spec, ver=ver), rd1_en=True
        ).sha(ver)
        for ver in ("v3", "v4")
    }
    op = DveOp(name, spec, subdim=False, uops_sha=shas)
    dve_ops.OPS.append(op)
    dve_ops.CUSTOM_DVE_SPECS[name] = spec
    dve_ops._SUB_OPCODE_FOR_NAME[name] = row
    return op


# ----------------------------------------------------------------- kernel --- #
_CACHE: dict = {}


def _build():
    if "nc" in _CACHE:
        return _CACHE["nc"]
    argmax_op = _make_argmax_op()

    nc = bacc.Bacc("TRN2", target_bir_lowering=False, debug=False)
    # chunk q of xThl: cols [2*CW*q, 2*CW*q+CW) = xh, [2*CW*q+CW, ...) = xl
    xThl_ext = nc.dram_tensor(
        "xThl", [D, 2 * L], mybir.dt.float16, kind="ExternalInput"
    )
    cbh_ext = nc.dram_tensor("cbh", [D, K], mybir.dt.float16, kind="ExternalInput")
    cbl_ext = nc.dram_tensor("cbl", [D, K], mybir.dt.float16, kind="ExternalInput")
    cbsq_ext = nc.dram_tensor(
        "cbsq", [1, K], mybir.dt.float32, kind="ExternalInput"
    )
    cbT_ext = nc.dram_tensor("cbT", [K, D], mybir.dt.float32, kind="ExternalInput")
    # idx, transposed: [p, t] = index of token t*TILE+p
    idxT_ext = nc.dram_tensor("idxT", [TILE, NT], mybir.dt.int32, kind="ExternalOutput")
    # xq, permuted: [p, g, j*D + d] = token (g*GRP+j)*TILE + p
    xqP_ext = nc.dram_tensor(
        "xqP", [TILE, NG, GRP * D], mybir.dt.float32, kind="ExternalOutput"
    )

    GW = GRP * TILE       # tokens per store group (1024)
    CT = 4                # tiles per load chunk
    CW = CT * TILE        # tokens per load chunk (512)

    with tile.TileContext(nc) as tc:
        with (
            tc.tile_pool(name="static", bufs=1) as static_pool,
            tc.tile_pool(name="loads", bufs=8) as load_pool,
            tc.tile_pool(name="work", bufs=2) as work_pool,
            tc.tile_pool(name="small", bufs=8) as small_pool,
            tc.tile_pool(name="out", bufs=3) as out_pool,
            tc.tile_pool(name="psum", bufs=3, space="PSUM") as psum_pool,
            tc.tile_pool(name="psum1", bufs=1, space="PSUM") as psum1_pool,
        ):
            cbh_t = static_pool.tile([D, K], mybir.dt.float16)
            cbl_t = static_pool.tile([D, K], mybir.dt.float16)
            cbsq_row = static_pool.tile([1, K], mybir.dt.float32)
            ones_t = static_pool.tile([1, TILE], mybir.dt.float32)
            cbsq_t = static_pool.tile([128, K], mybir.dt.float32)
            idx_acc = static_pool.tile([TILE, NT], mybir.dt.int32)
            nc.sync.dma_start(out=cbsq_row[:], in_=cbsq_ext[:])
            nc.sync.dma_start(out=cbh_t[:], in_=cbh_ext[:])

            # x loads: tiny first chunk (1 tile) so compute starts early,
            # then 4-tile chunks; all resident (bufs=8), all on the scalar
            # HWDGE queue, issued before anything else enters that FIFO.
            bounds = [0, TILE] + [CW * q for q in range(1, L // CW)] + [L]
            xchunks = []
            for q in range(len(bounds) - 1):
                lo, hi = bounds[q], bounds[q + 1]
                xThl_c = load_pool.tile([D, 2 * (hi - lo)], mybir.dt.float16,
                                        tag=f"xc{q}")
                nc.scalar.dma_start(
                    out=xThl_c[:], in_=xThl_ext[:, 2 * lo : 2 * hi]
                )
                xchunks.append((lo, hi, xThl_c))
                if q == 0:
                    # cbl rides the scalar queue right after the tiny first
                    # x chunk, in parallel with cbh on the sync queue
                    nc.scalar.dma_start(out=cbl_t[:], in_=cbl_ext[:])

            # broadcast cbsq to all 128 partitions: ones.T @ cbsq_row via PE,
            # PSUM->SBUF copy on the DVE (idle until the first argmax anyway)
            nc.vector.memset(ones_t[:], 1.0)
            cbsq_ps = psum1_pool.tile([128, K], mybir.dt.float32)
            # PE warm-up: ~3.5us of dummy matmuls with no DMA dependency so the
            # HAM clock-gate reaches 8/8 before the first real tile; results
            # land in the cbsq bank and are overwritten right after.
            wu_w = static_pool.tile([D, TILE], mybir.dt.float16)
            wu_m = static_pool.tile([D, 512], mybir.dt.float16)
            nc.vector.memset(wu_w[:], 0.0)
            nc.vector.memset(wu_m[:], 0.0)
            for _ in range(10):
                nc.tensor.matmul(
                    out=cbsq_ps[:, 0:512], lhsT=wu_w[:], rhs=wu_m[:],
                    start=True, stop=True,
                )
            for h in range(K // 512):
                hs = slice(h * 512, (h + 1) * 512)
                nc.tensor.matmul(
                    out=cbsq_ps[:, hs], lhsT=ones_t[:], rhs=cbsq_row[:, hs],
                    start=True, stop=True,
                )
            nc.vector.tensor_copy(out=cbsq_t[:], in_=cbsq_ps[:])

            def x_slices(t):
                for lo, hi, c in xchunks:
                    if lo <= t * TILE < hi:
                        off = t * TILE - lo
                        w = hi - lo
                        return (c[:, off : off + TILE],
                                c[:, w + off : w + off + TILE])
                raise AssertionError(t)

            for g in range(NG):
                xq_big = out_pool.tile([TILE, GRP, D], mybir.dt.float32)
                for j in range(GRP):
                    t = g * GRP + j
                    xh, xl = x_slices(t)
                    scores = psum_pool.tile([TILE, K], mybir.dt.float32)
                    # stationary operand changes once per tile: xh then xl
                    for h in range(K // 512):
                        hs = slice(h * 512, (h + 1) * 512)
                        nc.tensor.matmul(
                            out=scores[:, hs], lhsT=xh, rhs=cbh_t[:, hs],
                            start=True, stop=False,
                        )
                    for h in range(K // 512):
                        hs = slice(h * 512, (h + 1) * 512)
                        nc.tensor.matmul(
                            out=scores[:, hs], lhsT=xh, rhs=cbl_t[:, hs],
                            start=False, stop=False,
                        )
                    for h in range(K // 512):
                        hs = slice(h * 512, (h + 1) * 512)
                        nc.tensor.matmul(
                            out=scores[:, hs], lhsT=xl, rhs=cbh_t[:, hs],
                            start=False, stop=True,
                        )
                    scratch = work_pool.tile([TILE, K], mybir.dt.float32)
                    amax_f = small_pool.tile([TILE, 1], mybir.dt.float32)
                    nc.vector._custom_dve(
                        argmax_op,
                        out=scratch[:],
                        in0=scores[:],
                        in1=cbsq_t[:],
                        accum_out=amax_f[:],
                    )
                    # cast f32 -> i32 on the (mostly idle) scalar engine
                    nc.scalar.copy(out=idx_acc[:, t : t + 1], in_=amax_f[:])

                    nc.gpsimd.indirect_dma_start(
                        out=xq_big[:, j, :],
                        out_offset=None,
                        in_=cbT_ext[:],
                        in_offset=bass.IndirectOffsetOnAxis(
                            ap=idx_acc[:, t : t + 1], axis=0
                        ),
                    )
                    if g == NG - 1:
                        nc.sync.dma_start(
                            out=xqP_ext[:, g, j * D : (j + 1) * D],
                            in_=xq_big[:, j : j + 1, :],
                        )
                    elif j % 2 == 1:
                        nc.sync.dma_start(
                            out=xqP_ext[:, g, (j - 1) * D : (j + 1) * D],
                            in_=xq_big[:, j - 1 : j + 1, :],
                        )
            nc.sync.dma_start(out=idxT_ext[:], in_=idx_acc[:])

    nc.compile()
    _CACHE["nc"] = nc
    return nc


def _ensure_ntff_hook():
    """The image's antenv lacks axon_hooks; synthesize it so trace=True works."""
    try:
        from antenv.axon_hooks import get_axon_ntff_profile_hook  # noqa: F401

        return
    except ImportError:
        pass
    import types

    import antenv
    from trn_agent_boot.trn_boot import _ntff_profile_via_ctypes

    hook = _ntff_profile_via_ctypes("/opt/axon/libaxon_pjrt.so")
    mod = types.ModuleType("antenv.axon_hooks")
    mod._hook = hook
    mod.get_axon_ntff_profile_hook = lambda: mod._hook
    mod.set_axon_ntff_profile_hook = lambda h: setattr(mod, "_hook", h)
    sys.modules["antenv.axon_hooks"] = mod
    antenv.axon_hooks = mod


def _run(x: np.ndarray, codebook: np.ndarray, trace: bool = False):
    """Shard, run on 8 cores, reassemble. Returns (xq, idx, results_obj)."""
    if trace:
        _ensure_ntff_hook()
    nc = _build()
    x = np.ascontiguousarray(x, dtype=np.float32)
    codebook = np.ascontiguousarray(codebook, dtype=np.float32)
    cbs = codebook * np.float32(S)
    cbh = cbs.astype(np.float16)
    cbl = (cbs - cbh.astype(np.float32)).astype(np.float16)
    # scores come out scaled by S^2; match with S^2 * |c|^2 / 2
    cbsq_half = (cbs.astype(np.float64) ** 2).sum(axis=0) / 2.0
    cbsq_row = cbsq_half.astype(np.float32)[None, :]
    cbT = np.ascontiguousarray(codebook.T)  # unscaled: gather output values

    CW = 4 * TILE
    bounds = [0, TILE] + [CW * q for q in range(1, L // CW)] + [L]
    in_maps = []
    for c in range(N_CORES):
        xs = np.ascontiguousarray(x[c].T) * np.float32(S)
        xh = xs.astype(np.float16)
        xl = (xs - xh.astype(np.float32)).astype(np.float16)
        xThl = np.empty((D, 2 * L), dtype=np.float16)
        for q in range(len(bounds) - 1):
            lo, hi = bounds[q], bounds[q + 1]
            w = hi - lo
            xThl[:, 2 * lo : 2 * lo + w] = xh[:, lo:hi]
            xThl[:, 2 * lo + w : 2 * hi] = xl[:, lo:hi]
        in_maps.append(
            {
                "xThl": xThl,
                "cbh": cbh,
                "cbl": cbl,
                "cbsq": cbsq_row,
                "cbT": cbT,
            }
        )
    res = run_bass_kernel_spmd(nc, in_maps, list(range(N_CORES)), trace=trace)

    xq = np.empty((B, L, D), dtype=np.float32)
    idx = np.empty((B, L), dtype=np.int32)
    for c in range(N_CORES):
        idxT = res.results[c]["idxT"]              # [TILE, NT]
        idx[c] = idxT.T.reshape(L)
        xqP = res.results[c]["xqP"]                # [TILE, NG, GRP*D]
        xq[c] = (
            xqP.reshape(TILE, NG, GRP, D)
            .transpose(1, 2, 0, 3)
            .reshape(L, D)
        )
    return xq, idx, res


def kernel(x: np.ndarray, codebook: np.ndarray):
    xq, idx, _ = _run(x, codebook, trace=False)
    x = np.asarray(x, dtype=np.float32)
    # commitment loss: mean((stop_grad(xq) - x)^2), accumulated in f64
    diff = xq - x
    inner_loss = np.float32(np.mean(diff.astype(np.float64) ** 2))
    # straight-through estimator: x + stop_grad(xq - x) — reproduce the
    # reference's forward arithmetic exactly
    x_quantized = x + diff
    return x_quantized, idx, inner_loss
